# revision 7
# baseline (speedup 1.0000x reference)
"""Trainium2 Bass kernel for BalancedIPRMPNN (GNN message passing).

Reference computation (G=128 disjoint graphs, NPG=512 nodes each, H=128):
    h2   = x @ (W_emb @ W_gcn) + b_emb @ W_gcn          # embedding+GCN linear folded
    m    = relu(D^-1/2 (Adj + I) D^-1/2 @ h2 + b_gcn)   # GCN propagate (per graph)
    virt = einsum('gnv,gnh->gvh', edge_weights, m)      # weighted pooling (V=64)
    t1   = relu(virt @ vW1 + vb1)
    gf   = mean_v(t1 @ vW2 + vb2)
    out  = relu(gf @ mW1 + mb1) @ mW2 + mb2             # [G, 10]

Strategy: data-parallel over graphs, 16 graphs per core on 8 cores.  The
message passing runs as a dense per-graph [512,512] adjacency matmul on the
tensor engine; host folds the symmetric degree normalization into x (rows
pre-scaled by dinv) and edge_weights (rows pre-scaled by dinv), ships the
integer adjacency (exact in fp16) transposed for the lhsT layout.  The MLP
tail runs batched across graphs in a transposed layout so every bias is a
per-partition ScalarE activation bias.
"""

import numpy as np

import concourse.bass as bass
import concourse.mybir as mybir
import concourse.tile as tile
from concourse import bacc
from concourse.bass_utils import run_bass_kernel_spmd

# Problem constants (hardcoded per contract)
G, NPG, H, IN, OUT, V = 128, 512, 128, 128, 10, 64
N = G * NPG
N_CORES = 8
GPC = G // N_CORES          # graphs per core = 16
NS = GPC * NPG              # nodes per core  = 8192
KB = NPG // 128             # 4 k-blocks of 128 nodes per graph

F32 = mybir.dt.float32
F32R = mybir.dt.float32r
F16 = mybir.dt.float16

_CACHE = {}


def _build_program(with_bias: bool):
    """Build the per-core Bass/Tile program (identical on all 8 cores)."""
    nc = bacc.Bacc("TRN2", target_bir_lowering=False)

    # ---- DRAM I/O ----
    xsT = nc.dram_tensor("xsT", [IN, NS], F32, kind="ExternalInput")          # dinv-scaled x, transposed
    W1 = nc.dram_tensor("W1", [IN, H], F32, kind="ExternalInput")             # W_emb @ W_gcn
    # adjacency counts (+I), pre-arranged to SBUF layout: [g, p, kb*NPG + d]
    adjT = nc.dram_tensor("adjT", [GPC, 128, KB * NPG], F16, kind="ExternalInput")
    # dinv-scaled edge_weights, pre-arranged: [g, p, kb*V + v]
    ews = nc.dram_tensor("ews", [GPC, 128, KB * V], F16, kind="ExternalInput")
    vW1 = nc.dram_tensor("vW1", [H, H], F32, kind="ExternalInput")
    vb1 = nc.dram_tensor("vb1", [H, 1], F32, kind="ExternalInput")
    vW2s = nc.dram_tensor("vW2s", [H, H], F32, kind="ExternalInput")          # vW2 / V
    vb2 = nc.dram_tensor("vb2", [H, 1], F32, kind="ExternalInput")
    mW1 = nc.dram_tensor("mW1", [H, H], F32, kind="ExternalInput")
    mb1 = nc.dram_tensor("mb1", [H, 1], F32, kind="ExternalInput")
    mW2 = nc.dram_tensor("mW2", [H, OUT], F32, kind="ExternalInput")
    mb2 = nc.dram_tensor("mb2", [OUT, 1], F32, kind="ExternalInput")
    if with_bias:
        biasL = nc.dram_tensor("biasL", [GPC, 2, NPG], F16, kind="ExternalInput")
        biasR = nc.dram_tensor("biasR", [2, H], F16, kind="ExternalInput")
    outT = nc.dram_tensor("outT", [OUT, GPC], F32, kind="ExternalOutput")

    with tile.TileContext(nc) as tc:
        with (
            tc.tile_pool(name="consts", bufs=1) as consts,
            tc.tile_pool(name="xchunk", bufs=3) as xchunk_pool,
            tc.tile_pool(name="upool", bufs=GPC) as u_pool,
            tc.tile_pool(name="adj", bufs=3) as adj_pool,
            tc.tile_pool(name="ewsp", bufs=3) as ews_pool,
            tc.tile_pool(name="mp", bufs=2) as m_pool,
            tc.tile_pool(name="blp", bufs=3) as bl_pool,
            tc.tile_pool(name="ph2", bufs=2, space="PSUM") as ph2,
            tc.tile_pool(name="pm", bufs=3, space="PSUM") as pm,
            tc.tile_pool(name="pv", bufs=2, space="PSUM") as pv,
            tc.tile_pool(name="pd", bufs=1, space="PSUM") as pd,
        ):
            # ---- load constants ----
            W1_sb = consts.tile([IN, H], F32)
            nc.sync.dma_start(out=W1_sb[:], in_=W1[:])
            vW1_sb = consts.tile([H, H], F32)
            nc.sync.dma_start(out=vW1_sb[:], in_=vW1[:])
            vW2_sb = consts.tile([H, H], F32)
            nc.sync.dma_start(out=vW2_sb[:], in_=vW2s[:])
            mW1_sb = consts.tile([H, H], F32)
            nc.sync.dma_start(out=mW1_sb[:], in_=mW1[:])
            mW2_sb = consts.tile([H, OUT], F32)
            nc.sync.dma_start(out=mW2_sb[:], in_=mW2[:])
            vb1_sb = consts.tile([H, 1], F32)
            nc.sync.dma_start(out=vb1_sb[:], in_=vb1[:])
            vb2_sb = consts.tile([H, 1], F32)
            nc.sync.dma_start(out=vb2_sb[:], in_=vb2[:])
            mb1_sb = consts.tile([H, 1], F32)
            nc.sync.dma_start(out=mb1_sb[:], in_=mb1[:])
            mb2_sb = consts.tile([OUT, 1], F32)
            nc.sync.dma_start(out=mb2_sb[:], in_=mb2[:])
            if with_bias:
                biasR_sb = consts.tile([2, H], F16)
                nc.sync.dma_start(out=biasR_sb[:], in_=biasR[:])

            virtT = consts.tile([H, GPC * V], F32)  # virt^T, all graphs side by side

            us = []
            for g in range(GPC):
                # ---- embed: u = (dinv*x) @ W1, cast fp16, for this graph's 512 nodes ----
                xc = xchunk_pool.tile([IN, NPG], F32, tag="xc")
                nc.sync.dma_start(out=xc[:], in_=xsT[:, g * NPG:(g + 1) * NPG])
                u_g = u_pool.tile([128, KB * H], F16, tag="u")
                us.append(u_g)
                for kb in range(KB):
                    p_h2 = ph2.tile([128, H], F32, tag="ph2")
                    nc.tensor.matmul(
                        p_h2[:], xc[:, kb * 128:(kb + 1) * 128], W1_sb[:],
                        start=True, stop=True,
                    )
                    nc.vector.tensor_copy(out=u_g[:, kb * H:(kb + 1) * H], in_=p_h2[:])

                # ---- GCN propagate: m = relu(adjT.T @ u [+ bias]) ----
                adj_sb = adj_pool.tile([128, KB * NPG], F16, tag="adj")
                nc.sync.dma_start(out=adj_sb[:], in_=adjT[g])
                ews_sb = ews_pool.tile([128, KB * V], F16, tag="ews")
                nc.sync.dma_start(out=ews_sb[:], in_=ews[g])
                if with_bias:
                    bl_sb = bl_pool.tile([2, NPG], F16, tag="bl")
                    nc.sync.dma_start(out=bl_sb[:], in_=biasL[g])

                m_sb = m_pool.tile([128, KB * H], F16, tag="m")
                for mb in range(KB):
                    p_m = pm.tile([128, H], F32, tag="pm")
                    if with_bias:
                        nc.tensor.matmul(
                            p_m[:], bl_sb[:, mb * 128:(mb + 1) * 128], biasR_sb[:],
                            start=True, stop=False,
                        )
                    for kb in range(KB):
                        nc.tensor.matmul(
                            p_m[:],
                            adj_sb[:, kb * NPG + mb * 128: kb * NPG + (mb + 1) * 128],
                            u_g[:, kb * H:(kb + 1) * H],
                            start=(kb == 0 and not with_bias),
                            stop=(kb == KB - 1),
                        )
                    nc.scalar.activation(
                        out=m_sb[:, mb * H:(mb + 1) * H], in_=p_m[:],
                        func=mybir.ActivationFunctionType.Relu,
                    )

                # ---- pooling: virtT[:, g] = m^T @ ews ----
                p_v = pv.tile([128, V], F32, tag="pv")
                for kb in range(KB):
                    nc.tensor.matmul(
                        p_v[:],
                        m_sb[:, kb * H:(kb + 1) * H],
                        ews_sb[:, kb * V:(kb + 1) * V],
                        start=(kb == 0), stop=(kb == KB - 1),
                    )
                nc.vector.tensor_copy(out=virtT[:, g * V:(g + 1) * V], in_=p_v[:])

            # ---- MLP tail (batched over graphs, transposed chain) ----
            t1 = consts.tile([H, GPC * V], F32)
            for half in range(2):
                p_t1 = pd.tile([128, 512], F32, tag="pd")
                nc.tensor.matmul(
                    p_t1[:],
                    vW1_sb[:],
                    virtT[:, half * 512:(half + 1) * 512],
                    start=True, stop=True,
                )
                nc.scalar.activation(
                    out=t1[:, half * 512:(half + 1) * 512], in_=p_t1[:],
                    func=mybir.ActivationFunctionType.Relu, bias=vb1_sb[:],
                )
            t1s = consts.tile([H, GPC], F32)
            nc.vector.tensor_reduce(
                out=t1s[:], in_=t1[:].rearrange("p (g v) -> p g v", v=V),
                axis=mybir.AxisListType.X, op=mybir.AluOpType.add,
            )
            p_gf = pd.tile([128, GPC], F32, tag="pd")
            nc.tensor.matmul(
                p_gf[:], vW2_sb[:], t1s[:],
                start=True, stop=True,
            )
            gf = consts.tile([H, GPC], F32)
            nc.scalar.activation(
                out=gf[:], in_=p_gf[:],
                func=mybir.ActivationFunctionType.Identity, bias=vb2_sb[:],
            )
            p_q1 = pd.tile([128, GPC], F32, tag="pd")
            nc.tensor.matmul(
                p_q1[:], mW1_sb[:], gf[:],
                start=True, stop=True,
            )
            q1 = consts.tile([H, GPC], F32)
            nc.scalar.activation(
                out=q1[:], in_=p_q1[:],
                func=mybir.ActivationFunctionType.Relu, bias=mb1_sb[:],
            )
            p_o = pd.tile([OUT, GPC], F32, tag="pd")
            nc.tensor.matmul(
                p_o[:], mW2_sb[:], q1[:],
                start=True, stop=True,
            )
            o_sb = consts.tile([OUT, GPC], F32)
            nc.scalar.activation(
                out=o_sb[:], in_=p_o[:],
                func=mybir.ActivationFunctionType.Identity, bias=mb2_sb[:],
            )
            nc.sync.dma_start(out=outT[:], in_=o_sb[:])

    nc.finalize()
    return nc


def _reference_numpy(x, edge_index, W_emb, b_emb, W_gcn, b_gcn, edge_weights,
                     vW1, vb1, vW2, vb2, mW1, mb1, mW2, mb2):
    """Pure-numpy fallback (used only if graphs are not disjoint)."""
    src, dst = edge_index[0].astype(np.int64), edge_index[1].astype(np.int64)
    h = x @ W_emb + b_emb
    h2 = h @ W_gcn
    deg = np.bincount(dst, minlength=N).astype(np.float32) + 1.0
    dinv = 1.0 / np.sqrt(deg)
    m = np.zeros_like(h2)
    np.add.at(m, dst, h2[src] * (dinv[src] * dinv[dst])[:, None])
    m += h2 * (dinv * dinv)[:, None]
    m = np.maximum(m + b_gcn, 0.0)
    hg = m.reshape(G, NPG, -1)
    virt = np.einsum('gnv,gnh->gvh', edge_weights, hg)
    t1 = np.maximum(virt @ vW1 + vb1, 0.0) @ vW2 + vb2
    gf = t1.mean(axis=1)
    return np.maximum(gf @ mW1 + mb1, 0.0) @ mW2 + mb2


def kernel(x, edge_index, batch, W_emb, b_emb, W_gcn, b_gcn, edge_weights,
           vW1, vb1, vW2, vb2, mW1, mb1, mW2, mb2, _trace=False):
    x = np.asarray(x, dtype=np.float32)
    edge_index = np.asarray(edge_index, dtype=np.int32)
    W_emb = np.asarray(W_emb, dtype=np.float32)
    b_emb = np.asarray(b_emb, dtype=np.float32)
    W_gcn = np.asarray(W_gcn, dtype=np.float32)
    b_gcn = np.asarray(b_gcn, dtype=np.float32)
    edge_weights = np.asarray(edge_weights, dtype=np.float32)
    vW1, vb1 = np.asarray(vW1, np.float32), np.asarray(vb1, np.float32)
    vW2, vb2 = np.asarray(vW2, np.float32), np.asarray(vb2, np.float32)
    mW1, mb1 = np.asarray(mW1, np.float32), np.asarray(mb1, np.float32)
    mW2, mb2 = np.asarray(mW2, np.float32), np.asarray(mb2, np.float32)

    src = edge_index[0].astype(np.int64)
    dst = edge_index[1].astype(np.int64)
    if not np.array_equal(src // NPG, dst // NPG):
        # cross-graph edges: dense per-graph adjacency doesn't apply
        return _reference_numpy(x, edge_index, W_emb, b_emb, W_gcn, b_gcn,
                                edge_weights, vW1, vb1, vW2, vb2, mW1, mb1,
                                mW2, mb2).astype(np.float32)

    # ---- host prep ----
    deg = (np.bincount(dst, minlength=N) + 1).astype(np.float32)  # in-degree + self loop
    dinv = (1.0 / np.sqrt(deg)).astype(np.float32)

    # per-graph transposed adjacency counts (+ self loops), exact small ints in fp16
    gidx = src // NPG
    lin = (gidx * NPG + (src % NPG)) * NPG + (dst % NPG)
    counts = np.bincount(lin, minlength=G * NPG * NPG)
    adjT_all = counts.reshape(G, NPG, NPG).astype(np.float16)
    diag = np.arange(NPG)
    adjT_all[:, diag, diag] += np.float16(1.0)
    # SBUF layout: [g, p, kb*NPG + d] so each per-graph DMA is [128, contiguous]
    adjT_sb_all = np.ascontiguousarray(
        adjT_all.reshape(G, KB, 128, NPG).transpose(0, 2, 1, 3).reshape(G, 128, KB * NPG)
    )

    xs = (x * dinv[:, None])  # fold D^-1/2 into x rows
    ews_all = (edge_weights * dinv.reshape(G, NPG, 1)).astype(np.float16)
    ews_sb_all = np.ascontiguousarray(
        ews_all.reshape(G, KB, 128, V).transpose(0, 2, 1, 3).reshape(G, 128, KB * V)
    )

    W1h = (W_emb @ W_gcn).astype(np.float32)
    vW2s_h = (vW2 / np.float32(V)).astype(np.float32)
    bvec = (b_emb @ W_gcn).astype(np.float32)
    with_bias = bool(np.any(bvec) or np.any(b_gcn))
    if with_bias:
        # m-psum bias = wvec ⊗ bvec + sqrt(deg) ⊗ b_gcn, with
        # wvec = (Adj+I) @ dinv per graph (host-computable rank-2 correction)
        dinv_g = dinv.reshape(G, NPG)
        wvec = np.einsum('gsd,gs->gd', adjT_all.astype(np.float32), dinv_g)
        sdeg = np.sqrt(deg).reshape(G, NPG)
        biasL_all = np.stack([wvec, sdeg], axis=1).astype(np.float16)  # [G, 2, NPG]
        biasR_np = np.stack([bvec, b_gcn], axis=0).astype(np.float16)  # [2, H]

    key = with_bias
    if key not in _CACHE:
        _CACHE[key] = _build_program(with_bias)
    nc = _CACHE[key]

    in_maps = []
    for c in range(N_CORES):
        gs = slice(c * GPC, (c + 1) * GPC)
        ns = slice(c * NS, (c + 1) * NS)
        im = {
            "xsT": np.ascontiguousarray(xs[ns].T),
            "W1": W1h,
            "adjT": adjT_sb_all[gs],
            "ews": ews_sb_all[gs],
            "vW1": vW1, "vb1": vb1.reshape(H, 1),
            "vW2s": vW2s_h, "vb2": vb2.reshape(H, 1),
            "mW1": mW1, "mb1": mb1.reshape(H, 1),
            "mW2": mW2, "mb2": mb2.reshape(OUT, 1),
        }
        if with_bias:
            im["biasL"] = np.ascontiguousarray(biasL_all[gs])
            im["biasR"] = biasR_np
        in_maps.append(im)

    res = run_bass_kernel_spmd(
        nc, in_maps, core_ids=list(range(N_CORES)), trace=_trace,
    )
    out = np.concatenate([res.results[c]["outT"].T for c in range(N_CORES)], axis=0)
    if _trace:
        kernel.last_exec_time_ns = res.exec_time_ns
        kernel.last_results = res
    return out.astype(np.float32)


# revision 8
# speedup vs baseline: 1.1522x; 1.1522x over previous
"""Trainium2 Bass kernel for BalancedIPRMPNN (GNN message passing).

Reference computation (G=128 disjoint graphs, NPG=512 nodes each, H=128):
    h2   = x @ (W_emb @ W_gcn) + b_emb @ W_gcn          # embedding+GCN linear folded
    m    = relu(D^-1/2 (Adj + I) D^-1/2 @ h2 + b_gcn)   # GCN propagate (per graph)
    virt = einsum('gnv,gnh->gvh', edge_weights, m)      # weighted pooling (V=64)
    t1   = relu(virt @ vW1 + vb1)
    gf   = mean_v(t1 @ vW2 + vb2)
    out  = relu(gf @ mW1 + mb1) @ mW2 + mb2             # [G, 10]

Strategy: data-parallel over graphs, 16 graphs per core on 8 cores.  The
message passing runs as a dense per-graph [512,512] adjacency matmul on the
tensor engine; host folds the symmetric degree normalization into x (rows
pre-scaled by dinv) and edge_weights (rows pre-scaled by dinv), ships the
integer adjacency (exact in fp16) transposed for the lhsT layout.  The MLP
tail runs batched across graphs in a transposed layout so every bias is a
per-partition ScalarE activation bias.
"""

import numpy as np

import concourse.bass as bass
import concourse.mybir as mybir
import concourse.tile as tile
from concourse import bacc
from concourse.bass_utils import run_bass_kernel_spmd

# Problem constants (hardcoded per contract)
G, NPG, H, IN, OUT, V = 128, 512, 128, 128, 10, 64
N = G * NPG
N_CORES = 8
GPC = G // N_CORES          # graphs per core = 16
NS = GPC * NPG              # nodes per core  = 8192
KB = NPG // 128             # 4 k-blocks of 128 nodes per graph

F32 = mybir.dt.float32
F16 = mybir.dt.float16

X_FP16 = True               # ship x in fp16 (halves x DMA, 4x faster h2 matmuls)

_CACHE = {}


def _build_program(with_bias: bool):
    """Build the per-core Bass/Tile program (identical on all 8 cores)."""
    nc = bacc.Bacc("TRN2", target_bir_lowering=False)
    XDT = F16 if X_FP16 else F32

    # ---- DRAM I/O ----
    xsT = nc.dram_tensor("xsT", [IN, NS], XDT, kind="ExternalInput")          # dinv-scaled x, transposed
    W1 = nc.dram_tensor("W1", [IN, H], XDT, kind="ExternalInput")             # W_emb @ W_gcn
    # adjacency counts (+I), pre-arranged to SBUF layout: [g, p, kb*NPG + d]
    adjT = nc.dram_tensor("adjT", [GPC, 128, KB * NPG], F16, kind="ExternalInput")
    # dinv-scaled edge_weights, pre-arranged: [g, p, kb*V + v]
    ews = nc.dram_tensor("ews", [GPC, 128, KB * V], F16, kind="ExternalInput")
    vW1 = nc.dram_tensor("vW1", [H, H], F32, kind="ExternalInput")
    vb1 = nc.dram_tensor("vb1", [H, 1], F32, kind="ExternalInput")
    vW2s = nc.dram_tensor("vW2s", [H, H], F32, kind="ExternalInput")          # vW2 / V
    vb2 = nc.dram_tensor("vb2", [H, 1], F32, kind="ExternalInput")
    mW1 = nc.dram_tensor("mW1", [H, H], F32, kind="ExternalInput")
    mb1 = nc.dram_tensor("mb1", [H, 1], F32, kind="ExternalInput")
    mW2 = nc.dram_tensor("mW2", [H, OUT], F32, kind="ExternalInput")
    mb2 = nc.dram_tensor("mb2", [OUT, 1], F32, kind="ExternalInput")
    if with_bias:
        biasL = nc.dram_tensor("biasL", [GPC, 2, NPG], F16, kind="ExternalInput")
        biasR = nc.dram_tensor("biasR", [2, H], F16, kind="ExternalInput")
    outT = nc.dram_tensor("outT", [OUT, GPC], F32, kind="ExternalOutput")

    with tile.TileContext(nc) as tc:
        with (
            tc.tile_pool(name="consts", bufs=1) as consts,
            tc.tile_pool(name="xchunk", bufs=4) as xchunk_pool,
            tc.tile_pool(name="upool", bufs=2) as u_pool,
            tc.tile_pool(name="adj", bufs=4) as adj_pool,
            tc.tile_pool(name="ewsp", bufs=4) as ews_pool,
            tc.tile_pool(name="mp", bufs=2) as m_pool,
            tc.tile_pool(name="blp", bufs=3) as bl_pool,
            tc.tile_pool(name="ph2", bufs=2, space="PSUM") as ph2,
            tc.tile_pool(name="pm", bufs=3, space="PSUM") as pm,
            tc.tile_pool(name="pv", bufs=2, space="PSUM") as pv,
            tc.tile_pool(name="pd", bufs=1, space="PSUM") as pd,
        ):
            # critical-path data first: graph 0's inputs, then W1
            xc0 = xchunk_pool.tile([IN, NPG], XDT, tag="xc")
            nc.sync.dma_start(out=xc0[:], in_=xsT[:, 0:NPG])
            W1_sb = consts.tile([IN, H], XDT)
            nc.sync.dma_start(out=W1_sb[:], in_=W1[:])
            adj0 = adj_pool.tile([128, KB * NPG], F16, tag="adj")
            nc.gpsimd.dma_start(out=adj0[:], in_=adjT[0])
            ews0 = ews_pool.tile([128, KB * V], F16, tag="ews")
            nc.sync.dma_start(out=ews0[:], in_=ews[0])

            vW1_sb = consts.tile([H, H], F32)
            nc.scalar.dma_start(out=vW1_sb[:], in_=vW1[:])
            vW2_sb = consts.tile([H, H], F32)
            nc.scalar.dma_start(out=vW2_sb[:], in_=vW2s[:])
            mW1_sb = consts.tile([H, H], F32)
            nc.scalar.dma_start(out=mW1_sb[:], in_=mW1[:])
            mW2_sb = consts.tile([H, OUT], F32)
            nc.scalar.dma_start(out=mW2_sb[:], in_=mW2[:])
            vb1_sb = consts.tile([H, 1], F32)
            nc.scalar.dma_start(out=vb1_sb[:], in_=vb1[:])
            vb2_sb = consts.tile([H, 1], F32)
            nc.scalar.dma_start(out=vb2_sb[:], in_=vb2[:])
            mb1_sb = consts.tile([H, 1], F32)
            nc.scalar.dma_start(out=mb1_sb[:], in_=mb1[:])
            mb2_sb = consts.tile([OUT, 1], F32)
            nc.scalar.dma_start(out=mb2_sb[:], in_=mb2[:])
            if with_bias:
                biasR_sb = consts.tile([2, H], F16)
                nc.scalar.dma_start(out=biasR_sb[:], in_=biasR[:])

            virtT = consts.tile([H, GPC * V], F32)  # virt^T, all graphs side by side
            t1 = consts.tile([H, GPC * V], F32)
            t1s = consts.tile([H, GPC], F32)

            for g in range(GPC):
                # ---- embed: u = (dinv*x) @ W1, cast fp16, for this graph's 512 nodes ----
                if g == 0:
                    xc = xc0
                else:
                    xc = xchunk_pool.tile([IN, NPG], XDT, tag="xc")
                    nc.sync.dma_start(out=xc[:], in_=xsT[:, g * NPG:(g + 1) * NPG])
                u_g = u_pool.tile([128, KB * H], F16, tag="u")
                for kb in range(KB):
                    p_h2 = ph2.tile([128, H], F32, tag="ph2")
                    nc.tensor.matmul(
                        p_h2[:], xc[:, kb * 128:(kb + 1) * 128], W1_sb[:],
                        start=True, stop=True,
                    )
                    nc.vector.tensor_copy(out=u_g[:, kb * H:(kb + 1) * H], in_=p_h2[:])

                # ---- GCN propagate: m = relu(adjT.T @ u [+ bias]) ----
                if g == 0:
                    adj_sb, ews_sb = adj0, ews0
                else:
                    adj_sb = adj_pool.tile([128, KB * NPG], F16, tag="adj")
                    nc.gpsimd.dma_start(out=adj_sb[:], in_=adjT[g])
                    ews_sb = ews_pool.tile([128, KB * V], F16, tag="ews")
                    nc.sync.dma_start(out=ews_sb[:], in_=ews[g])
                if with_bias:
                    bl_sb = bl_pool.tile([2, NPG], F16, tag="bl")
                    nc.sync.dma_start(out=bl_sb[:], in_=biasL[g])

                m_sb = m_pool.tile([128, KB * H], F16, tag="m")
                for mb in range(KB):
                    p_m = pm.tile([128, H], F32, tag="pm")
                    if with_bias:
                        nc.tensor.matmul(
                            p_m[:], bl_sb[:, mb * 128:(mb + 1) * 128], biasR_sb[:],
                            start=True, stop=False,
                        )
                    for kb in range(KB):
                        nc.tensor.matmul(
                            p_m[:],
                            adj_sb[:, kb * NPG + mb * 128: kb * NPG + (mb + 1) * 128],
                            u_g[:, kb * H:(kb + 1) * H],
                            start=(kb == 0 and not with_bias),
                            stop=(kb == KB - 1),
                        )
                    nc.scalar.activation(
                        out=m_sb[:, mb * H:(mb + 1) * H], in_=p_m[:],
                        func=mybir.ActivationFunctionType.Relu,
                    )

                # ---- pooling: virtT[:, g] = m^T @ ews ----
                p_v = pv.tile([128, V], F32, tag="pv")
                for kb in range(KB):
                    nc.tensor.matmul(
                        p_v[:],
                        m_sb[:, kb * H:(kb + 1) * H],
                        ews_sb[:, kb * V:(kb + 1) * V],
                        start=(kb == 0), stop=(kb == KB - 1),
                    )
                nc.vector.tensor_copy(out=virtT[:, g * V:(g + 1) * V], in_=p_v[:])

                # ---- MLP first stage per half as soon as its 8 graphs are done ----
                if g in (GPC // 2 - 1, GPC - 1):
                    half = 0 if g == GPC // 2 - 1 else 1
                    p_t1 = pd.tile([128, 512], F32, tag="pd")
                    nc.tensor.matmul(
                        p_t1[:], vW1_sb[:], virtT[:, half * 512:(half + 1) * 512],
                        start=True, stop=True,
                    )
                    nc.scalar.activation(
                        out=t1[:, half * 512:(half + 1) * 512], in_=p_t1[:],
                        func=mybir.ActivationFunctionType.Relu, bias=vb1_sb[:],
                    )
                    nc.vector.tensor_reduce(
                        out=t1s[:, half * 8:(half + 1) * 8],
                        in_=t1[:, half * 512:(half + 1) * 512]
                            .rearrange("p (g v) -> p g v", v=V),
                        axis=mybir.AxisListType.X, op=mybir.AluOpType.add,
                    )

            # ---- rest of the MLP tail ----
            p_gf = pd.tile([128, GPC], F32, tag="pd")
            nc.tensor.matmul(p_gf[:], vW2_sb[:], t1s[:], start=True, stop=True)
            gf = consts.tile([H, GPC], F32)
            nc.scalar.activation(
                out=gf[:], in_=p_gf[:],
                func=mybir.ActivationFunctionType.Identity, bias=vb2_sb[:],
            )
            p_q1 = pd.tile([128, GPC], F32, tag="pd")
            nc.tensor.matmul(p_q1[:], mW1_sb[:], gf[:], start=True, stop=True)
            q1 = consts.tile([H, GPC], F32)
            nc.scalar.activation(
                out=q1[:], in_=p_q1[:],
                func=mybir.ActivationFunctionType.Relu, bias=mb1_sb[:],
            )
            p_o = pd.tile([OUT, GPC], F32, tag="pd")
            nc.tensor.matmul(p_o[:], mW2_sb[:], q1[:], start=True, stop=True)
            o_sb = consts.tile([OUT, GPC], F32)
            nc.scalar.activation(
                out=o_sb[:], in_=p_o[:],
                func=mybir.ActivationFunctionType.Identity, bias=mb2_sb[:],
            )
            nc.sync.dma_start(out=outT[:], in_=o_sb[:])

    nc.finalize()
    return nc


def _reference_numpy(x, edge_index, W_emb, b_emb, W_gcn, b_gcn, edge_weights,
                     vW1, vb1, vW2, vb2, mW1, mb1, mW2, mb2):
    """Pure-numpy fallback (used only if graphs are not disjoint)."""
    src, dst = edge_index[0].astype(np.int64), edge_index[1].astype(np.int64)
    h = x @ W_emb + b_emb
    h2 = h @ W_gcn
    deg = np.bincount(dst, minlength=N).astype(np.float32) + 1.0
    dinv = 1.0 / np.sqrt(deg)
    m = np.zeros_like(h2)
    np.add.at(m, dst, h2[src] * (dinv[src] * dinv[dst])[:, None])
    m += h2 * (dinv * dinv)[:, None]
    m = np.maximum(m + b_gcn, 0.0)
    hg = m.reshape(G, NPG, -1)
    virt = np.einsum('gnv,gnh->gvh', edge_weights, hg)
    t1 = np.maximum(virt @ vW1 + vb1, 0.0) @ vW2 + vb2
    gf = t1.mean(axis=1)
    return np.maximum(gf @ mW1 + mb1, 0.0) @ mW2 + mb2


def kernel(x, edge_index, batch, W_emb, b_emb, W_gcn, b_gcn, edge_weights,
           vW1, vb1, vW2, vb2, mW1, mb1, mW2, mb2, _trace=False):
    x = np.asarray(x, dtype=np.float32)
    edge_index = np.asarray(edge_index, dtype=np.int32)
    W_emb = np.asarray(W_emb, dtype=np.float32)
    b_emb = np.asarray(b_emb, dtype=np.float32)
    W_gcn = np.asarray(W_gcn, dtype=np.float32)
    b_gcn = np.asarray(b_gcn, dtype=np.float32)
    edge_weights = np.asarray(edge_weights, dtype=np.float32)
    vW1, vb1 = np.asarray(vW1, np.float32), np.asarray(vb1, np.float32)
    vW2, vb2 = np.asarray(vW2, np.float32), np.asarray(vb2, np.float32)
    mW1, mb1 = np.asarray(mW1, np.float32), np.asarray(mb1, np.float32)
    mW2, mb2 = np.asarray(mW2, np.float32), np.asarray(mb2, np.float32)

    src = edge_index[0].astype(np.int64)
    dst = edge_index[1].astype(np.int64)
    if not np.array_equal(src // NPG, dst // NPG):
        # cross-graph edges: dense per-graph adjacency doesn't apply
        return _reference_numpy(x, edge_index, W_emb, b_emb, W_gcn, b_gcn,
                                edge_weights, vW1, vb1, vW2, vb2, mW1, mb1,
                                mW2, mb2).astype(np.float32)

    # ---- host prep ----
    deg = (np.bincount(dst, minlength=N) + 1).astype(np.float32)  # in-degree + self loop
    dinv = (1.0 / np.sqrt(deg)).astype(np.float32)

    # per-graph transposed adjacency counts (+ self loops), exact small ints in fp16
    gidx = src // NPG
    lin = (gidx * NPG + (src % NPG)) * NPG + (dst % NPG)
    counts = np.bincount(lin, minlength=G * NPG * NPG)
    adjT_all = counts.reshape(G, NPG, NPG).astype(np.float16)
    diag = np.arange(NPG)
    adjT_all[:, diag, diag] += np.float16(1.0)
    # SBUF layout: [g, p, kb*NPG + d] so each per-graph DMA is [128, contiguous]
    adjT_sb_all = np.ascontiguousarray(
        adjT_all.reshape(G, KB, 128, NPG).transpose(0, 2, 1, 3).reshape(G, 128, KB * NPG)
    )

    xdt = np.float16 if X_FP16 else np.float32
    xs = (x * dinv[:, None])  # fold D^-1/2 into x rows
    xsT_np = np.ascontiguousarray(xs.T.astype(xdt))  # [IN, N]
    ews_all = (edge_weights * dinv.reshape(G, NPG, 1)).astype(np.float16)
    ews_sb_all = np.ascontiguousarray(
        ews_all.reshape(G, KB, 128, V).transpose(0, 2, 1, 3).reshape(G, 128, KB * V)
    )

    W1h = (W_emb @ W_gcn).astype(xdt)
    vW2s_h = (vW2 / np.float32(V)).astype(np.float32)
    bvec = (b_emb @ W_gcn).astype(np.float32)
    with_bias = bool(np.any(bvec) or np.any(b_gcn))
    if with_bias:
        # m-psum bias = wvec ⊗ bvec + sqrt(deg) ⊗ b_gcn, with
        # wvec = (Adj+I) @ dinv per graph (host-computable rank-2 correction)
        dinv_g = dinv.reshape(G, NPG)
        wvec = np.einsum('gsd,gs->gd', adjT_all.astype(np.float32), dinv_g)
        sdeg = np.sqrt(deg).reshape(G, NPG)
        biasL_all = np.stack([wvec, sdeg], axis=1).astype(np.float16)  # [G, 2, NPG]
        biasR_np = np.stack([bvec, b_gcn], axis=0).astype(np.float16)  # [2, H]

    key = with_bias
    if key not in _CACHE:
        _CACHE[key] = _build_program(with_bias)
    nc = _CACHE[key]

    in_maps = []
    for c in range(N_CORES):
        gs = slice(c * GPC, (c + 1) * GPC)
        im = {
            "xsT": np.ascontiguousarray(xsT_np[:, c * NS:(c + 1) * NS]),
            "W1": W1h,
            "adjT": adjT_sb_all[gs],
            "ews": ews_sb_all[gs],
            "vW1": vW1, "vb1": vb1.reshape(H, 1),
            "vW2s": vW2s_h, "vb2": vb2.reshape(H, 1),
            "mW1": mW1, "mb1": mb1.reshape(H, 1),
            "mW2": mW2, "mb2": mb2.reshape(OUT, 1),
        }
        if with_bias:
            im["biasL"] = np.ascontiguousarray(biasL_all[gs])
            im["biasR"] = biasR_np
        in_maps.append(im)

    res = run_bass_kernel_spmd(
        nc, in_maps, core_ids=list(range(N_CORES)), trace=_trace,
    )
    out = np.concatenate([res.results[c]["outT"].T for c in range(N_CORES)], axis=0)
    if _trace:
        kernel.last_exec_time_ns = res.exec_time_ns
        kernel.last_results = res
    return out.astype(np.float32)
